# revision 1
# baseline (speedup 1.0000x reference)
"""Cross-attention Trainium2 kernel (Bass/Tile), data-parallel over batch on 8 cores.

Reference computation per batch b (C=256, CR=64, N=H*W=4096):
    Q = Wq @ src          [CR, N]
    K = Wk @ gui          [CR, N]
    V = Wv @ gui + bv     [C, N]
    energy[n, m] = sum_q Q[q, n] K[q, m]
    attn = softmax_m(energy)
    out = gamma * (V @ attn^T) + src

Kernel strategy (per core, one batch item):
    - compute energy TRANSPOSED: eT[m, n] = sum_q K[q, m] Q[q, n] so the
      unnormalized attention tiles come out of the PE in exactly the [m, n]
      orientation the V @ attn^T matmul needs as its moving operand.
    - exp on ScalarE (no max subtraction: |energy| <= ~6 at these scales so
      exp stays well inside fp32 range; equals softmax up to fp32 rounding).
    - row sums via a ones-matmul (sum over the partition dim on the PE),
      replicated across all 128 partitions so the final normalization is a
      plain elementwise multiply.
    - normalization, gamma and residual folded into the PSUM->SBUF drain.
    - matmul operands in bf16 (full PE rate + fast weight load); every
      operand already passes through a DVE/ACT drain, so the conversions are
      free. PSUM accumulation stays fp32. The residual path keeps the
      original fp32 `source`.
    - Q/K live duplicated on partitions 0-63 / 64-127 so the K=64 energy
      matmuls run pairwise-concurrent in the two PE row-group halves.
"""

from contextlib import ExitStack

import numpy as np

import concourse.bacc as bacc
import concourse.bass as bass
import concourse.mybir as mybir
import concourse.tile as tile
from concourse.bass_utils import run_bass_kernel_spmd
from concourse.masks import make_identity

B, C, H, W = 8, 256, 64, 64
N = H * W            # 4096 pixels
CR = 64              # reduced channels for Q/K
N_CORES = 8
NT = 512             # n-chunk (query) tile
NCH = N // NT        # 8
MT = 128             # m-chunk (key) tile: PE output partition max
MCH = N // MT        # 32
CCH = C // 128       # 2 channel chunks

F32 = mybir.dt.float32
BF16 = mybir.dt.bfloat16
EXP = mybir.ActivationFunctionType.Exp

ts = bass.ts

ROW_TILE = True  # pairwise-concurrent energy matmuls in PE row-group halves


def build_kernel(loop=1):
    """Build + compile the single-core program (SPMD across 8 cores).

    loop > 1 unrolls the whole kernel body that many times in one NEFF; used
    by test.py to measure marginal (steady-state) HW time per execution.
    """
    nc = bacc.Bacc("TRN2", target_bir_lowering=False, debug=False)

    src_d = nc.dram_tensor("source", [C, N], F32, kind="ExternalInput").ap()
    gui_d = nc.dram_tensor("guidance", [C, N], F32, kind="ExternalInput").ap()
    wq_d = nc.dram_tensor("Wq", [CR, C], F32, kind="ExternalInput").ap()
    wk_d = nc.dram_tensor("Wk", [CR, C], F32, kind="ExternalInput").ap()
    wv_d = nc.dram_tensor("Wv", [C, C], F32, kind="ExternalInput").ap()
    bv_d = nc.dram_tensor("bv", [C], F32, kind="ExternalInput").ap()
    g_d = nc.dram_tensor("gamma", [1], F32, kind="ExternalInput").ap()
    out_d = nc.dram_tensor("out", [C, N], F32, kind="ExternalOutput").ap()

    with tile.TileContext(nc) as tc:
        for it in range(loop):
            with ExitStack() as ctx:
                _body(ctx, tc, src_d, gui_d, wq_d, wk_d, wv_d, bv_d, g_d,
                      out_d, sfx=f"_{it}")
    nc.compile()
    return nc


def _body(ctx, tc, src_d, gui_d, wq_d, wk_d, wv_d, bv_d, g_d, out_d, sfx=""):
    nc = tc.nc

    consts = ctx.enter_context(tc.tile_pool(name="consts" + sfx, bufs=1))
    big = ctx.enter_context(tc.tile_pool(name="big" + sfx, bufs=1))

    # ---- persistent SBUF tensors ----
    src_sb = big.tile([128, CCH, N], F32)    # fp32, for the residual
    src_bf = big.tile([128, CCH, N], BF16)   # bf16 matmul operand copy
    gui_bf = big.tile([128, CCH, N], BF16)
    # Q/K with q duplicated onto partitions 64..127 for PE row-tiling.
    QQ = big.tile([128, N], BF16)
    KK = big.tile([128, N], BF16)
    VT = big.tile([128, MCH, C], BF16)       # [m%128, m//128, c] = V^T

    # ---- weights / constants ----
    wq_sb = consts.tile([CR, C], F32)
    wk_sb = consts.tile([CR, C], F32)
    wv_sb = consts.tile([128, CCH, C], F32)  # [c%128, c//128, ch]
    bv_sb = consts.tile([1, C], BF16)
    g128 = consts.tile([128, 1], F32)
    ones = consts.tile([128, 128], BF16)
    ident = consts.tile([128, 128], F32)

    nc.sync.dma_start(out=wq_sb[:], in_=wq_d)
    nc.sync.dma_start(out=wk_sb[:], in_=wk_d)
    wv_r = wv_d.rearrange("(t p) c -> t p c", p=128)
    for t in range(CCH):
        nc.sync.dma_start(out=wv_sb[:, t, :], in_=wv_r[t])
    bv_f = consts.tile([1, C], F32)
    nc.sync.dma_start(out=bv_f[:], in_=bv_d.unsqueeze(0))
    nc.vector.tensor_copy(bv_sb[:], bv_f[:])
    nc.sync.dma_start(out=g128[:], in_=g_d.to_broadcast([128, 1]))
    nc.vector.memset(ones[:], 1.0)
    make_identity(nc, ident[:])

    # ---- load activations; keep fp32 source, bf16 copies for matmuls ----
    src_r = src_d.rearrange("(t p) n -> t p n", p=128)
    gui_r = gui_d.rearrange("(t p) n -> t p n", p=128)
    with tc.tile_pool(name="stage" + sfx, bufs=1) as stage:
        gui_f = stage.tile([128, CCH, N], F32)
        for t in range(CCH):
            nc.sync.dma_start(out=src_sb[:, t, :], in_=src_r[t])
            nc.sync.dma_start(out=gui_f[:, t, :], in_=gui_r[t])
        nc.vector.tensor_copy(src_bf[:], src_sb[:])
        nc.vector.tensor_copy(gui_bf[:], gui_f[:])

    # ---- transpose weights on the PE (fp32 has no DMA transpose) ----
    # wqt2/wkt2: [c-chunk 128, q duplicated to 128]; wvt: [ch, c] = Wv^T
    wqt2 = consts.tile([128, CCH, 128], BF16)
    wkt2 = consts.tile([128, CCH, 128], BF16)
    wvt = consts.tile([128, CCH, C], BF16)

    with tc.tile_pool(name="tp_psum" + sfx, bufs=2, space=bass.MemorySpace.PSUM) as tpp:
        for t in range(CCH):
            for w_sb, w_t2 in ((wq_sb, wqt2), (wk_sb, wkt2)):
                p = tpp.tile([128, CR], F32, tag="tp")
                nc.tensor.transpose(p[:], w_sb[:, ts(t, 128)], ident[:CR, :CR])
                nc.vector.tensor_copy(w_t2[:, t, 0:CR], p[:])
                nc.vector.tensor_copy(w_t2[:, t, CR:128], p[:])
            for j in range(CCH):
                # wvt[:, t, j*128:+128] = Wv[j*128:+128, t*128:+128]^T
                p = tpp.tile([128, 128], F32, tag="tp")
                nc.tensor.transpose(p[:], wv_sb[:, j, ts(t, 128)], ident[:])
                nc.vector.tensor_copy(wvt[:, t, ts(j, 128)], p[:])

    # ---- projections ----
    with tc.tile_pool(name="proj_psum" + sfx, bufs=4, space=bass.MemorySpace.PSUM) as pp:
        for i in range(NCH):
            qp = pp.tile([128, NT], F32, tag="qk")
            for t in range(CCH):
                nc.tensor.matmul(qp[:], wqt2[:, t, :], src_bf[:, t, ts(i, NT)],
                                 start=(t == 0), stop=(t == CCH - 1))
            nc.vector.tensor_copy(QQ[:, ts(i, NT)], qp[:])
            kp = pp.tile([128, NT], F32, tag="qk")
            for t in range(CCH):
                nc.tensor.matmul(kp[:], wkt2[:, t, :], gui_bf[:, t, ts(i, NT)],
                                 start=(t == 0), stop=(t == CCH - 1))
            nc.vector.tensor_copy(KK[:, ts(i, NT)], kp[:])
        for j in range(MCH):
            vp = pp.tile([128, C], F32, tag="v")
            # bias row via K=1 ones-matmul: vp[m, c] = bv[c]
            nc.tensor.matmul(vp[:], ones[0:1, :], bv_sb[:], start=True, stop=False)
            for t in range(CCH):
                nc.tensor.matmul(vp[:], gui_bf[:, t, ts(j, MT)], wvt[:, t, :],
                                 start=False, stop=(t == CCH - 1))
            nc.vector.tensor_copy(VT[:, j, :], vp[:])

    # ---- attention main loop ----
    e_ps = ctx.enter_context(
        tc.tile_pool(name="e_psum" + sfx, bufs=3, space=bass.MemorySpace.PSUM))
    o_ps = ctx.enter_context(
        tc.tile_pool(name="o_psum" + sfx, bufs=3, space=bass.MemorySpace.PSUM))
    s_ps = ctx.enter_context(
        tc.tile_pool(name="s_psum" + sfx, bufs=2, space=bass.MemorySpace.PSUM))
    e_sb = ctx.enter_context(tc.tile_pool(name="e_sb" + sfx, bufs=4))
    fin = ctx.enter_context(tc.tile_pool(name="fin" + sfx, bufs=2))
    o_sb = ctx.enter_context(tc.tile_pool(name="o_sb" + sfx, bufs=4))

    out_r = out_d.rearrange("(t p) n -> t p n", p=128)

    for i in range(NCH):
        o0 = o_ps.tile([128, NT], F32, tag="o")
        o1 = o_ps.tile([128, NT], F32, tag="o")
        sm = s_ps.tile([128, NT], F32, tag="s")

        def energy(j):
            b0 = CR * (j % 2) if ROW_TILE else 0
            ep = e_ps.tile([128, NT], F32, tag="e")
            nc.tensor.matmul(ep[:], KK[b0:b0 + CR, ts(j, MT)],
                             QQ[b0:b0 + CR, ts(i, NT)],
                             start=True, stop=True, tile_position=(b0, 0))
            return ep

        ep = energy(0)
        for j in range(MCH):
            ee = e_sb.tile([128, NT], BF16, tag="ee")
            nc.scalar.activation(ee[:], ep[:], EXP)
            if j + 1 < MCH:
                ep = energy(j + 1)  # keep PE one tile ahead of ACT
            first, last = j == 0, j == MCH - 1
            nc.tensor.matmul(o0[:], VT[:, j, 0:128], ee[:],
                             start=first, stop=last)
            nc.tensor.matmul(o1[:], VT[:, j, 128:256], ee[:],
                             start=first, stop=last)
            nc.tensor.matmul(sm[:], ones[:], ee[:], start=first, stop=last)

        # out = o * (gamma / sum) + src
        rsg = fin.tile([128, NT], F32, tag="rsg")
        nc.vector.reciprocal(rsg[:], sm[:])
        nc.vector.tensor_scalar_mul(rsg[:], rsg[:], g128[:])
        for t, op in enumerate((o0, o1)):
            ot = o_sb.tile([128, NT], F32, tag="ot")
            nc.vector.tensor_mul(ot[:], op[:], rsg[:])
            nc.vector.tensor_add(ot[:], ot[:], src_sb[:, t, ts(i, NT)])
            nc.sync.dma_start(out=out_r[t][:, ts(i, NT)], in_=ot[:])


_NC_CACHE = []


def _get_nc():
    if not _NC_CACHE:
        _NC_CACHE.append(build_kernel())
    return _NC_CACHE[0]


def make_in_maps(**inputs):
    f = lambda a: np.ascontiguousarray(np.asarray(a, dtype=np.float32))
    src = f(inputs["source"]).reshape(B, C, N)
    gui = f(inputs["guidance"]).reshape(B, C, N)
    shared = {
        "Wq": f(inputs["Wq"]),
        "Wk": f(inputs["Wk"]),
        "Wv": f(inputs["Wv"]),
        "bv": f(inputs["bv"]),
        "gamma": f(inputs["gamma"]),
    }
    return [dict(source=src[b], guidance=gui[b], **shared) for b in range(B)]


def kernel(**inputs) -> np.ndarray:
    nc = _get_nc()
    res = run_bass_kernel_spmd(nc, make_in_maps(**inputs),
                               core_ids=list(range(N_CORES)))
    out = np.stack([res.results[b]["out"] for b in range(B)])
    return out.reshape(B, C, H, W).astype(np.float32)



# revision 5
# speedup vs baseline: 1.6883x; 1.6883x over previous
"""Cross-attention Trainium2 kernel (Bass/Tile), data-parallel over batch on 8 cores.

Reference computation per batch b (C=256, CR=64, N=H*W=4096):
    Q = Wq @ src          [CR, N]
    K = Wk @ gui          [CR, N]
    V = Wv @ gui + bv     [C, N]
    energy[n, m] = sum_q Q[q, n] K[q, m]
    attn = softmax_m(energy)
    out = gamma * (V @ attn^T) + src

Kernel strategy (per core, one batch item):
    - energy computed TRANSPOSED: eT[m, n] = sum_q K[q, m] Q[q, n] so the
      unnormalized attention tiles come out of the PE in exactly the [m, n]
      orientation the V @ attn^T matmul needs as its moving operand.
    - m-tiles processed in PAIRS: two bf16 energy matmuls (PE row-group
      halves) write the two banks of one [128, 2, NT] PSUM tile; a single
      ACT instruction computes exp over both banks (halves ACT instruction
      count); the exp output is fp8e4 with bias -EXP_SHIFT (max energy is
      ~5.5, exp(e-1) <= 86 stays under the TRN fp8e4 max-normal 240).
    - output + denominator matmuls run in fp8 MatmulPerfMode.DoubleRow over
      the pair: lhsT [128, 2, 128] (VT pair / ones), rhs = the [128, 2, NT]
      fp8 exp tile -> 2x PE throughput on the dominant matmuls. The constant
      exp shift cancels in softmax (numerator and denominator both scaled).
    - V and the softmax denominator ones operand are fp8; quantization error
      averages out over 4096 diffuse attention weights (measured rel err
      ~7e-4 vs fp32 reference at gamma=1.7).
    - row sums via a DoubleRow ones-matmul replicated across all 128
      partitions so the final normalization is a plain elementwise multiply.
    - normalization, gamma and residual folded into the PSUM->SBUF drain.
      The residual path keeps the original fp32 `source`.
    - Q/K live duplicated on partitions 0-63 / 64-127 so the two bf16 energy
      matmuls of a pair run pairwise-concurrent in the PE row-group halves.
"""

from contextlib import ExitStack

import numpy as np

import concourse.bacc as bacc
import concourse.bass as bass
import concourse.mybir as mybir
import concourse.tile as tile
from concourse.bass_utils import run_bass_kernel_spmd
from concourse.masks import make_identity

B, C, H, W = 8, 256, 64, 64
N = H * W            # 4096 pixels
CR = 64              # reduced channels for Q/K
N_CORES = 8
NT = 512             # n-chunk (query) tile
NCH = N // NT        # 8
MT = 128             # m-chunk (key) tile: PE output partition max
MCH = N // MT        # 32
MPAIR = MCH // 2     # 16 m-tile pairs (fp8 DoubleRow granularity)
CCH = C // 128       # 2 channel chunks
EXP_SHIFT = 1.0      # exp(e - shift): keeps fp8 exp outputs <= ~86 < 240

F32 = mybir.dt.float32
BF16 = mybir.dt.bfloat16
F8 = mybir.dt.float8e4
EXP = mybir.ActivationFunctionType.Exp
DR = mybir.MatmulPerfMode.DoubleRow

ts = bass.ts


def build_kernel(loop=1):
    """Build + compile the single-core program (SPMD across 8 cores).

    loop > 1 unrolls the whole kernel body that many times in one NEFF; used
    by test.py to measure marginal (steady-state) HW time per execution.
    """
    nc = bacc.Bacc("TRN2", target_bir_lowering=False, debug=False)

    src_d = nc.dram_tensor("source", [C, N], F32, kind="ExternalInput").ap()
    gui_d = nc.dram_tensor("guidance", [C, N], F32, kind="ExternalInput").ap()
    wq_d = nc.dram_tensor("Wq", [CR, C], F32, kind="ExternalInput").ap()
    wk_d = nc.dram_tensor("Wk", [CR, C], F32, kind="ExternalInput").ap()
    wv_d = nc.dram_tensor("Wv", [C, C], F32, kind="ExternalInput").ap()
    bv_d = nc.dram_tensor("bv", [C], F32, kind="ExternalInput").ap()
    g_d = nc.dram_tensor("gamma", [1], F32, kind="ExternalInput").ap()
    out_d = nc.dram_tensor("out", [C, N], F32, kind="ExternalOutput").ap()

    with tile.TileContext(nc) as tc:
        for it in range(loop):
            with ExitStack() as ctx:
                _body(ctx, tc, src_d, gui_d, wq_d, wk_d, wv_d, bv_d, g_d,
                      out_d, sfx=f"_{it}")
    nc.compile()
    return nc


def _body(ctx, tc, src_d, gui_d, wq_d, wk_d, wv_d, bv_d, g_d, out_d, sfx=""):
    nc = tc.nc

    consts = ctx.enter_context(tc.tile_pool(name="consts" + sfx, bufs=1))
    big = ctx.enter_context(tc.tile_pool(name="big" + sfx, bufs=1))

    # ---- persistent SBUF tensors ----
    src_sb = big.tile([128, CCH, N], F32)    # fp32, for the residual
    src_bf = big.tile([128, CCH, N], BF16)   # bf16 matmul operand copy
    gui_bf = big.tile([128, CCH, N], BF16)
    # Q/K with q duplicated onto partitions 64..127 for PE row-tiling.
    QQ = big.tile([128, N], BF16)
    KK = big.tile([128, N], BF16)
    VT = big.tile([128, MCH, C], F8)         # [m%128, m//128, c] = V^T, fp8

    # ---- weights / constants ----
    wq_sb = consts.tile([CR, C], F32)
    wk_sb = consts.tile([CR, C], F32)
    wv_sb = consts.tile([128, CCH, C], F32)  # [c%128, c//128, ch]
    bv_sb = consts.tile([1, C], BF16)
    g128 = consts.tile([128, 1], F32)
    ones = consts.tile([1, MT], BF16)        # bias-row matmul operand
    ones8 = consts.tile([128, 2, MT], F8)    # fp8 DoubleRow row-sum operand
    ident = consts.tile([128, 128], F32)
    shift = consts.tile([128, 1], F32)       # exp bias: -EXP_SHIFT

    nc.sync.dma_start(out=wq_sb[:], in_=wq_d)
    nc.sync.dma_start(out=wk_sb[:], in_=wk_d)
    wv_r = wv_d.rearrange("(t p) c -> t p c", p=128)
    for t in range(CCH):
        nc.sync.dma_start(out=wv_sb[:, t, :], in_=wv_r[t])
    bv_f = consts.tile([1, C], F32)
    nc.sync.dma_start(out=bv_f[:], in_=bv_d.unsqueeze(0))
    nc.vector.tensor_copy(bv_sb[:], bv_f[:])
    nc.sync.dma_start(out=g128[:], in_=g_d.to_broadcast([128, 1]))
    nc.vector.memset(ones[:], 1.0)
    nc.vector.memset(ones8[:], 1.0)
    nc.vector.memset(shift[:], -EXP_SHIFT)
    make_identity(nc, ident[:])

    # ---- load activations; keep fp32 source, bf16 copies for matmuls ----
    src_r = src_d.rearrange("(t p) n -> t p n", p=128)
    gui_r = gui_d.rearrange("(t p) n -> t p n", p=128)
    with tc.tile_pool(name="stage" + sfx, bufs=1) as stage:
        gui_f = stage.tile([128, CCH, N], F32)
        for t in range(CCH):
            nc.sync.dma_start(out=src_sb[:, t, :], in_=src_r[t])
            nc.sync.dma_start(out=gui_f[:, t, :], in_=gui_r[t])
        nc.vector.tensor_copy(src_bf[:], src_sb[:])
        nc.vector.tensor_copy(gui_bf[:], gui_f[:])

    # ---- transpose weights on the PE (fp32 has no DMA transpose) ----
    # wqt2/wkt2: [c-chunk 128, q duplicated to 128]; wvt: [ch, c] = Wv^T
    wqt2 = consts.tile([128, CCH, 128], BF16)
    wkt2 = consts.tile([128, CCH, 128], BF16)
    wvt = consts.tile([128, CCH, C], BF16)

    with tc.tile_pool(name="tp_psum" + sfx, bufs=2, space=bass.MemorySpace.PSUM) as tpp:
        for t in range(CCH):
            for w_sb, w_t2 in ((wq_sb, wqt2), (wk_sb, wkt2)):
                p = tpp.tile([128, CR], F32, tag="tp")
                nc.tensor.transpose(p[:], w_sb[:, ts(t, 128)], ident[:CR, :CR])
                nc.vector.tensor_copy(w_t2[:, t, 0:CR], p[:])
                nc.vector.tensor_copy(w_t2[:, t, CR:128], p[:])
            for j in range(CCH):
                # wvt[:, t, j*128:+128] = Wv[j*128:+128, t*128:+128]^T
                p = tpp.tile([128, 128], F32, tag="tp")
                nc.tensor.transpose(p[:], wv_sb[:, j, ts(t, 128)], ident[:])
                nc.vector.tensor_copy(wvt[:, t, ts(j, 128)], p[:])

    # ---- projections ----
    with tc.tile_pool(name="proj_psum" + sfx, bufs=4, space=bass.MemorySpace.PSUM) as pp:
        for i in range(NCH):
            qp = pp.tile([128, NT], F32, tag="qk")
            for t in range(CCH):
                nc.tensor.matmul(qp[:], wqt2[:, t, :], src_bf[:, t, ts(i, NT)],
                                 start=(t == 0), stop=(t == CCH - 1))
            nc.vector.tensor_copy(QQ[:, ts(i, NT)], qp[:])
            kp = pp.tile([128, NT], F32, tag="qk")
            for t in range(CCH):
                nc.tensor.matmul(kp[:], wkt2[:, t, :], gui_bf[:, t, ts(i, NT)],
                                 start=(t == 0), stop=(t == CCH - 1))
            nc.vector.tensor_copy(KK[:, ts(i, NT)], kp[:])
        for j in range(MCH):
            vp = pp.tile([128, C], F32, tag="v")
            # bias row via K=1 ones-matmul: vp[m, c] = bv[c]
            nc.tensor.matmul(vp[:], ones[0:1, :], bv_sb[:], start=True, stop=False)
            for t in range(CCH):
                nc.tensor.matmul(vp[:], gui_bf[:, t, ts(j, MT)], wvt[:, t, :],
                                 start=False, stop=(t == CCH - 1))
            nc.vector.tensor_copy(VT[:, j, :], vp[:])

    # ---- attention main loop ----
    e_ps = ctx.enter_context(
        tc.tile_pool(name="e_psum" + sfx, bufs=2, space=bass.MemorySpace.PSUM))
    o_ps = ctx.enter_context(
        tc.tile_pool(name="o_psum" + sfx, bufs=3, space=bass.MemorySpace.PSUM))
    s_ps = ctx.enter_context(
        tc.tile_pool(name="s_psum" + sfx, bufs=1, space=bass.MemorySpace.PSUM))
    e_sb = ctx.enter_context(tc.tile_pool(name="e_sb" + sfx, bufs=4))
    fin = ctx.enter_context(tc.tile_pool(name="fin" + sfx, bufs=2))
    o_sb = ctx.enter_context(tc.tile_pool(name="o_sb" + sfx, bufs=4))

    out_r = out_d.rearrange("(t p) n -> t p n", p=128)

    for i in range(NCH):
        o0 = o_ps.tile([128, NT], F32, tag="o")
        o1 = o_ps.tile([128, NT], F32, tag="o")
        sm = s_ps.tile([128, NT], F32, tag="s")

        def energy_pair(jj):
            # two bf16 energy matmuls into one [128, 2, NT] PSUM pair tile,
            # row-group halves 0-63 / 64-127 for PE row-tiling concurrency
            ep = e_ps.tile([128, 2, NT], F32, tag="e")
            for h in range(2):
                b0 = CR * h
                nc.tensor.matmul(ep[:, h, :], KK[b0:b0 + CR, ts(2 * jj + h, MT)],
                                 QQ[b0:b0 + CR, ts(i, NT)],
                                 start=True, stop=True, tile_position=(b0, 0))
            return ep

        ep = energy_pair(0)
        for jj in range(MPAIR):
            ee = e_sb.tile([128, 2, NT], F8, tag="ee")
            nc.scalar.activation(ee[:], ep[:], EXP, bias=shift[:])
            if jj + 1 < MPAIR:
                ep = energy_pair(jj + 1)  # keep PE one pair ahead of ACT
            first, last = jj == 0, jj == MPAIR - 1
            vpair = VT[:, 2 * jj:2 * jj + 2, :]
            nc.tensor.matmul(o0[:], vpair[:, :, 0:128], ee[:],
                             start=first, stop=last, perf_mode=DR)
            nc.tensor.matmul(o1[:], vpair[:, :, 128:256], ee[:],
                             start=first, stop=last, perf_mode=DR)
            nc.tensor.matmul(sm[:], ones8[:], ee[:],
                             start=first, stop=last, perf_mode=DR)

        # out = o * (gamma / sum) + src
        rsg = fin.tile([128, NT], F32, tag="rsg")
        nc.vector.reciprocal(rsg[:], sm[:])
        nc.vector.tensor_scalar_mul(rsg[:], rsg[:], g128[:])
        for t, op in enumerate((o0, o1)):
            ot = o_sb.tile([128, NT], F32, tag="ot")
            nc.vector.tensor_mul(ot[:], op[:], rsg[:])
            nc.vector.tensor_add(ot[:], ot[:], src_sb[:, t, ts(i, NT)])
            nc.sync.dma_start(out=out_r[t][:, ts(i, NT)], in_=ot[:])


_NC_CACHE = []


def _get_nc():
    if not _NC_CACHE:
        _NC_CACHE.append(build_kernel())
    return _NC_CACHE[0]


def make_in_maps(**inputs):
    f = lambda a: np.ascontiguousarray(np.asarray(a, dtype=np.float32))
    src = f(inputs["source"]).reshape(B, C, N)
    gui = f(inputs["guidance"]).reshape(B, C, N)
    shared = {
        "Wq": f(inputs["Wq"]),
        "Wk": f(inputs["Wk"]),
        "Wv": f(inputs["Wv"]),
        "bv": f(inputs["bv"]),
        "gamma": f(inputs["gamma"]),
    }
    return [dict(source=src[b], guidance=gui[b], **shared) for b in range(B)]


def kernel(**inputs) -> np.ndarray:
    nc = _get_nc()
    res = run_bass_kernel_spmd(nc, make_in_maps(**inputs),
                               core_ids=list(range(N_CORES)))
    out = np.stack([res.results[b]["out"] for b in range(B)])
    return out.reshape(B, C, H, W).astype(np.float32)


# revision 10
# speedup vs baseline: 1.8199x; 1.0779x over previous
"""Cross-attention Trainium2 kernel (Bass/Tile), data-parallel over batch on 8 cores.

Reference computation per batch b (C=256, CR=64, N=H*W=4096):
    Q = Wq @ src          [CR, N]
    K = Wk @ gui          [CR, N]
    V = Wv @ gui + bv     [C, N]
    energy[n, m] = sum_q Q[q, n] K[q, m]
    attn = softmax_m(energy)
    out = gamma * (V @ attn^T) + src

Kernel strategy (per core, one batch item):
    - energy computed TRANSPOSED: eT[m, n] = sum_q K[q, m] Q[q, n] so the
      unnormalized attention tiles come out of the PE in exactly the [m, n]
      orientation the V @ attn^T matmul needs as its moving operand.
    - m-tiles processed in PAIRS: two bf16 energy matmuls (PE row-group
      halves) write the two banks of one [128, 2, NT] PSUM tile; a single
      ACT instruction computes exp over both banks (halves ACT instruction
      count); the exp output is fp8e4 with bias -EXP_SHIFT (max energy is
      ~5.5, exp(e-1) <= 86 stays under the TRN fp8e4 max-normal 240).
    - output + denominator matmuls run in fp8 MatmulPerfMode.DoubleRow over
      the pair: lhsT [128, 2, 128] (VT pair / ones), rhs = the [128, 2, NT]
      fp8 exp tile -> 2x PE throughput on the dominant matmuls. The constant
      exp shift cancels in softmax (numerator and denominator both scaled).
    - ACT exp is the steady-state bottleneck (~134 us busy), so the prologue
      is aggressively overlapped: src and gui stream over the two HWDGE
      queues (SP + ACT) in parallel, and all projections consume the fp32
      DMA data directly as float32r matmul operands (full PE rate at free
      dim >= 256, fp22 mantissa > bf16) -- no big DVE dtype-convert passes.
    - V bias folded into the V-projection PSUM drain as a DVE add against a
      partition-broadcast bv row (no bias matmuls).
    - Q projection runs just-in-time: chunk i+1 is projected inside the
      attention loop of chunk i, sharing a PSUM ring with the softmax
      denominator tile.
    - row sums via a DoubleRow ones-matmul replicated across all 128
      partitions so the final normalization is a plain elementwise multiply.
    - normalization, gamma and residual folded into the PSUM->SBUF drain.
      The residual path keeps the original fp32 `source`.
    - Q/K live duplicated on partitions 0-63 / 64-127 so the two bf16 energy
      matmuls of a pair run pairwise-concurrent in the PE row-group halves.
"""

from contextlib import ExitStack

import numpy as np

import concourse.bacc as bacc
import concourse.bass as bass
import concourse.mybir as mybir
import concourse.tile as tile
from concourse.bass_utils import run_bass_kernel_spmd
from concourse.masks import make_identity

B, C, H, W = 8, 256, 64, 64
N = H * W            # 4096 pixels
CR = 64              # reduced channels for Q/K
N_CORES = 8
NT = 512             # n-chunk (query) tile
NCH = N // NT        # 8
MT = 128             # m-chunk (key) tile: PE output partition max
MCH = N // MT        # 32
MPAIR = MCH // 2     # 16 m-tile pairs (fp8 DoubleRow granularity)
CCH = C // 128       # 2 channel chunks
EXP_SHIFT = 1.0      # exp(e - shift): keeps fp8 exp outputs <= ~86 < 240

F32 = mybir.dt.float32
F32R = mybir.dt.float32r
BF16 = mybir.dt.bfloat16
F8 = mybir.dt.float8e4
EXP = mybir.ActivationFunctionType.Exp
DR = mybir.MatmulPerfMode.DoubleRow

ts = bass.ts


def build_kernel(loop=1):
    """Build + compile the single-core program (SPMD across 8 cores).

    loop > 1 unrolls the whole kernel body that many times in one NEFF; used
    by test.py to measure marginal (steady-state) HW time per execution.
    """
    nc = bacc.Bacc("TRN2", target_bir_lowering=False, debug=False)

    src_d = nc.dram_tensor("source", [C, N], F32, kind="ExternalInput").ap()
    gui_d = nc.dram_tensor("guidance", [C, N], F32, kind="ExternalInput").ap()
    wq_d = nc.dram_tensor("Wq", [CR, C], F32, kind="ExternalInput").ap()
    wk_d = nc.dram_tensor("Wk", [CR, C], F32, kind="ExternalInput").ap()
    wv_d = nc.dram_tensor("Wv", [C, C], F32, kind="ExternalInput").ap()
    bv_d = nc.dram_tensor("bv", [C], F32, kind="ExternalInput").ap()
    g_d = nc.dram_tensor("gamma", [1], F32, kind="ExternalInput").ap()
    out_d = nc.dram_tensor("out", [C, N], F32, kind="ExternalOutput").ap()

    with tile.TileContext(nc) as tc:
        for it in range(loop):
            with ExitStack() as ctx:
                _body(ctx, tc, src_d, gui_d, wq_d, wk_d, wv_d, bv_d, g_d,
                      out_d, sfx=f"_{it}")
    nc.compile()
    return nc


def _body(ctx, tc, src_d, gui_d, wq_d, wk_d, wv_d, bv_d, g_d, out_d, sfx=""):
    nc = tc.nc

    consts = ctx.enter_context(tc.tile_pool(name="consts" + sfx, bufs=1))
    big = ctx.enter_context(tc.tile_pool(name="big" + sfx, bufs=1))

    # ---- persistent SBUF tensors ----
    # f32r-typed: DMA'd fp32 bits consumed directly by f32r matmuls (the
    # BIR verifier requires the memory location itself to be fp32r-typed)
    src_sb = big.tile([128, CCH, N], F32R)   # residual + Q-proj operand
    gui_sb = big.tile([128, CCH, N], F32R)   # K/V-proj operand
    # Q/K with q duplicated onto partitions 64..127 for PE row-tiling.
    QQ = big.tile([128, N], BF16)
    KK = big.tile([128, N], BF16)
    VT = big.tile([128, MCH, C], F8)         # [m%128, m//128, c] = V^T, fp8

    # ---- weights / constants ----
    wq_sb = consts.tile([CR, C], F32)
    wk_sb = consts.tile([CR, C], F32)
    wv_sb = consts.tile([128, CCH, C], F32)  # [c%128, c//128, ch]
    bv2 = consts.tile([128, 2, C], F32)      # bv broadcast, duplicated per pair
    g128 = consts.tile([128, 1], F32)
    ones8 = consts.tile([128, 2, MT], F8)    # fp8 DoubleRow row-sum operand
    ident = consts.tile([128, 128], F32)
    shift = consts.tile([128, 1], F32)       # exp bias: -EXP_SHIFT

    # Activations stream in 1024-column blocks on the SP HWDGE queue,
    # ordered by first use (DMA bandwidth is the serial resource): gui block
    # 0 feeds the first K/V projections, src block 0 the first Q projection,
    # later gui blocks feed the K chunks streamed inside the first attention
    # chunk. Small weights ride the otherwise-idle ACT HWDGE queue.
    gui_r = gui_d.rearrange("(t p) n -> t p n", p=128)
    src_r = src_d.rearrange("(t p) n -> t p n", p=128)
    NB = N // 4

    def block(dst, rr, b):
        for t in range(CCH):
            nc.sync.dma_start(out=dst[:, t, ts(b, NB)],
                              in_=rr[t][:, ts(b, NB)].bitcast(F32R))

    block(gui_sb, gui_r, 0)
    block(src_sb, src_r, 0)
    block(gui_sb, gui_r, 1)
    block(gui_sb, gui_r, 2)
    block(src_sb, src_r, 1)
    block(gui_sb, gui_r, 3)
    block(src_sb, src_r, 2)
    block(src_sb, src_r, 3)
    nc.scalar.dma_start(out=wq_sb[:], in_=wq_d)
    nc.scalar.dma_start(out=wk_sb[:], in_=wk_d)
    wv_r = wv_d.rearrange("(t p) c -> t p c", p=128)
    for t in range(CCH):
        nc.scalar.dma_start(out=wv_sb[:, t, :], in_=wv_r[t])
    for h in range(2):
        nc.scalar.dma_start(out=bv2[:, h, :], in_=bv_d.unsqueeze(0).to_broadcast([128, C]))
    nc.scalar.dma_start(out=g128[:], in_=g_d.to_broadcast([128, 1]))

    nc.vector.memset(ones8[:], 1.0)
    nc.vector.memset(shift[:], -EXP_SHIFT)
    make_identity(nc, ident[:])

    # ---- transpose weights on the PE (fp32, consumed as f32r) ----
    # wqt2/wkt2: [c-chunk 128, q duplicated to 128]; wvt: [ch, c] = Wv^T
    wqt2 = consts.tile([128, CCH, 128], F32R)
    wkt2 = consts.tile([128, CCH, 128], F32R)
    wvt = consts.tile([128, CCH, C], F32R)

    with tc.tile_pool(name="tp_psum" + sfx, bufs=2, space=bass.MemorySpace.PSUM) as tpp:
        for t in range(CCH):
            for w_sb, w_t2 in ((wk_sb, wkt2), (wq_sb, wqt2)):
                p = tpp.tile([128, CR], F32, tag="tp")
                nc.tensor.transpose(p[:], w_sb[:, ts(t, 128)], ident[:CR, :CR])
                nc.vector.tensor_copy(w_t2[:, t, 0:CR], p[:])
                nc.vector.tensor_copy(w_t2[:, t, CR:128], p[:])
            for j in range(CCH):
                # wvt[:, t, j*128:+128] = Wv[j*128:+128, t*128:+128]^T
                p = tpp.tile([128, 128], F32, tag="tp")
                nc.tensor.transpose(p[:], wv_sb[:, j, ts(t, 128)], ident[:])
                nc.vector.tensor_copy(wvt[:, t, ts(j, 128)], p[:])


    # s_ps is opened before the projection pool: the sm denominator ring is
    # also the Q-projection ring (alternating slots), and the c0 Q projection
    # is interleaved with the K/V projections below.
    s_ps = ctx.enter_context(
        tc.tile_pool(name="s_psum" + sfx, bufs=2, space=bass.MemorySpace.PSUM))

    # ---- prologue projections: K chunks 0-1 and Q chunk 0 only ----
    # Everything else (K chunks 2-7, all V pairs, Q chunks 1-7) streams
    # inside the attention loop so the first exp fires ~9us in, right after
    # the first DMA blocks land.
    qp0 = s_ps.tile([128, NT], F32, tag="s")
    with tc.tile_pool(name="proj_psum" + sfx, bufs=2, space=bass.MemorySpace.PSUM) as pp:
        for i in range(2):
            kp = pp.tile([128, NT], F32, tag="qk")
            for t in range(CCH):
                nc.tensor.matmul(kp[:], wkt2[:, t, :],
                                 gui_sb[:, t, ts(i, NT)],
                                 start=(t == 0), stop=(t == CCH - 1))
            nc.vector.tensor_copy(KK[:, ts(i, NT)], kp[:])
        for t in range(CCH):
            nc.tensor.matmul(qp0[:], wqt2[:, t, :],
                             src_sb[:, t, ts(0, NT)],
                             start=(t == 0), stop=(t == CCH - 1))
        nc.vector.tensor_copy(QQ[:, ts(0, NT)], qp0[:])

    # ---- attention main loop ----
    e_ps = ctx.enter_context(
        tc.tile_pool(name="e_psum" + sfx, bufs=2, space=bass.MemorySpace.PSUM))
    o_ps = ctx.enter_context(
        tc.tile_pool(name="o_psum" + sfx, bufs=2, space=bass.MemorySpace.PSUM))
    e_sb = ctx.enter_context(tc.tile_pool(name="e_sb" + sfx, bufs=4))
    fin = ctx.enter_context(tc.tile_pool(name="fin" + sfx, bufs=2))
    o_sb = ctx.enter_context(tc.tile_pool(name="o_sb" + sfx, bufs=4))

    out_r = out_d.rearrange("(t p) n -> t p n", p=128)

    def q_proj(i):
        qp = s_ps.tile([128, NT], F32, tag="s")
        for t in range(CCH):
            nc.tensor.matmul(qp[:], wqt2[:, t, :],
                             src_sb[:, t, ts(i, NT)],
                             start=(t == 0), stop=(t == CCH - 1))
        nc.vector.tensor_copy(QQ[:, ts(i, NT)], qp[:])

    def v_pair(jj):
        # V pair rides the e_ps ring: each half of the [128, 2, NT] tile is
        # one PSUM bank; one DVE drain per pair folds in the bias (fp8 out)
        vp = e_ps.tile([128, 2, NT], F32, tag="e")
        for h in range(2):
            for t in range(CCH):
                nc.tensor.matmul(vp[:, h, 0:C],
                                 gui_sb[:, t, ts(2 * jj + h, MT)],
                                 wvt[:, t, :],
                                 start=(t == 0), stop=(t == CCH - 1))
        nc.vector.tensor_add(VT[:, 2 * jj:2 * jj + 2, :], vp[:, :, 0:C], bv2[:])

    def k_proj(c):
        kp = e_ps.tile([128, 2, NT], F32, tag="e")
        for t in range(CCH):
            nc.tensor.matmul(kp[:, 0, :], wkt2[:, t, :],
                             gui_sb[:, t, ts(c, NT)],
                             start=(t == 0), stop=(t == CCH - 1))
        nc.vector.tensor_copy(KK[:, ts(c, NT)], kp[:, 0, :])

    v_pair(0)
    v_pair(1)

    for i in range(NCH):
        o0 = o_ps.tile([128, NT], F32, tag="o")
        o1 = o_ps.tile([128, NT], F32, tag="o")
        sm = s_ps.tile([128, NT], F32, tag="s")

        def energy_pair(jj):
            # two bf16 energy matmuls into one [128, 2, NT] PSUM pair tile,
            # row-group halves 0-63 / 64-127 for PE row-tiling concurrency
            ep = e_ps.tile([128, 2, NT], F32, tag="e")
            for h in range(2):
                b0 = CR * h
                nc.tensor.matmul(ep[:, h, :], KK[b0:b0 + CR, ts(2 * jj + h, MT)],
                                 QQ[b0:b0 + CR, ts(i, NT)],
                                 start=True, stop=True, tile_position=(b0, 0))
            return ep

        ep = energy_pair(0)
        for jj in range(MPAIR):
            ee = e_sb.tile([128, 2, NT], F8, tag="ee")
            nc.scalar.activation(ee[:], ep[:], EXP, bias=shift[:])
            if jj + 1 < MPAIR:
                ep = energy_pair(jj + 1)  # keep PE one pair ahead of ACT
            if i == 0:
                # stream the remaining projections through the first chunk
                if jj % 2 == 0 and jj // 2 + 2 < NCH:
                    k_proj(jj // 2 + 2)
                if jj + 2 < MPAIR:
                    v_pair(jj + 2)
            if jj == 8 and i + 1 < NCH:
                q_proj(i + 1)  # JIT projection of the next query chunk
            first, last = jj == 0, jj == MPAIR - 1
            vpair = VT[:, 2 * jj:2 * jj + 2, :]
            nc.tensor.matmul(o0[:], vpair[:, :, 0:128], ee[:],
                             start=first, stop=last, perf_mode=DR)
            nc.tensor.matmul(o1[:], vpair[:, :, 128:256], ee[:],
                             start=first, stop=last, perf_mode=DR)
            nc.tensor.matmul(sm[:], ones8[:], ee[:],
                             start=first, stop=last, perf_mode=DR)

        # out = o * (gamma / sum) + src
        rsg = fin.tile([128, NT], F32, tag="rsg")
        nc.vector.reciprocal(rsg[:], sm[:])
        nc.vector.tensor_scalar_mul(rsg[:], rsg[:], g128[:])
        # last chunk: strip-mine so the output DMAs overlap the DVE tail,
        # alternating HWDGE queues to pipeline the DMA issue cost
        nstrip = 2 if i == NCH - 1 else 1
        st = NT // nstrip
        for t, op in enumerate((o0, o1)):
            ot = o_sb.tile([128, NT], F32, tag="ot")
            for u in range(nstrip):
                sl = slice(u * st, (u + 1) * st)
                nc.vector.tensor_mul(ot[:, sl], op[:, sl], rsg[:, sl])
                nc.vector.tensor_add(ot[:, sl], ot[:, sl],
                                     src_sb[:, t, ts(i, NT)][:, sl].bitcast(F32))
                q = nc.scalar if (t + u) % 2 else nc.sync
                q.dma_start(out=out_r[t][:, ts(i, NT)][:, sl], in_=ot[:, sl])


_NC_CACHE = []


def _get_nc():
    if not _NC_CACHE:
        _NC_CACHE.append(build_kernel())
    return _NC_CACHE[0]


def make_in_maps(**inputs):
    f = lambda a: np.ascontiguousarray(np.asarray(a, dtype=np.float32))
    src = f(inputs["source"]).reshape(B, C, N)
    gui = f(inputs["guidance"]).reshape(B, C, N)
    shared = {
        "Wq": f(inputs["Wq"]),
        "Wk": f(inputs["Wk"]),
        "Wv": f(inputs["Wv"]),
        "bv": f(inputs["bv"]),
        "gamma": f(inputs["gamma"]),
    }
    return [dict(source=src[b], guidance=gui[b], **shared) for b in range(B)]


def kernel(**inputs) -> np.ndarray:
    nc = _get_nc()
    res = run_bass_kernel_spmd(nc, make_in_maps(**inputs),
                               core_ids=list(range(N_CORES)))
    out = np.stack([res.results[b]["out"] for b in range(B)])
    return out.reshape(B, C, H, W).astype(np.float32)


# revision 16
# speedup vs baseline: 1.8512x; 1.0172x over previous
"""Cross-attention Trainium2 kernel (Bass/Tile), data-parallel over batch on 8 cores.

Reference computation per batch b (C=256, CR=64, N=H*W=4096):
    Q = Wq @ src          [CR, N]
    K = Wk @ gui          [CR, N]
    V = Wv @ gui + bv     [C, N]
    energy[n, m] = sum_q Q[q, n] K[q, m]
    attn = softmax_m(energy)
    out = gamma * (V @ attn^T) + src

Kernel strategy (per core, one batch item):
    - energy computed TRANSPOSED: eT[m, n] = sum_q K[q, m] Q[q, n] so the
      unnormalized attention tiles come out of the PE in exactly the [m, n]
      orientation the V @ attn^T matmul needs as its moving operand.
    - m-tiles processed in PAIRS: two bf16 energy matmuls (PE row-group
      halves) write the two banks of one [128, 2, NT] PSUM tile; a single
      ACT instruction computes exp over both banks (halves ACT instruction
      count); the exp output is fp8e4 with bias -EXP_SHIFT (max energy is
      ~5.5, exp(e-1) <= 86 stays under the TRN fp8e4 max-normal 240).
    - output + denominator matmuls run in fp8 MatmulPerfMode.DoubleRow over
      the pair: lhsT [128, 2, 128] (VT pair / ones), rhs = the [128, 2, NT]
      fp8 exp tile -> 2x PE throughput on the dominant matmuls. The constant
      exp shift cancels in softmax (numerator and denominator both scaled).
    - ACT exp is the steady-state bottleneck (~134 us busy), so the prologue
      is aggressively overlapped: src and gui stream over the two HWDGE
      queues (SP + ACT) in parallel, and all projections consume the fp32
      DMA data directly as float32r matmul operands (full PE rate at free
      dim >= 256, fp22 mantissa > bf16) -- no big DVE dtype-convert passes.
    - V bias folded into the V-projection PSUM drain as a DVE add against a
      partition-broadcast bv row (no bias matmuls).
    - Q projection runs just-in-time: chunk i+1 is projected inside the
      attention loop of chunk i, sharing a PSUM ring with the softmax
      denominator tile.
    - row sums via a DoubleRow ones-matmul replicated across all 128
      partitions so the final normalization is a plain elementwise multiply.
    - normalization, gamma and residual folded into the PSUM->SBUF drain.
      The residual path keeps the original fp32 `source`.
    - Q/K live duplicated on partitions 0-63 / 64-127 so the two bf16 energy
      matmuls of a pair run pairwise-concurrent in the PE row-group halves.
"""

from contextlib import ExitStack

import numpy as np

import concourse.bacc as bacc
import concourse.bass as bass
import concourse.mybir as mybir
import concourse.tile as tile
from concourse.bass_utils import run_bass_kernel_spmd
from concourse.masks import make_identity

B, C, H, W = 8, 256, 64, 64
N = H * W            # 4096 pixels
CR = 64              # reduced channels for Q/K
N_CORES = 8
NT = 512             # n-chunk (query) tile
NCH = N // NT        # 8
MT = 128             # m-chunk (key) tile: PE output partition max
MCH = N // MT        # 32
MPAIR = MCH // 2     # 16 m-tile pairs (fp8 DoubleRow granularity)
CCH = C // 128       # 2 channel chunks
EXP_SHIFT = 1.0      # exp(e - shift): keeps fp8 exp outputs <= ~86 < 240

F32 = mybir.dt.float32
F32R = mybir.dt.float32r
BF16 = mybir.dt.bfloat16
F8 = mybir.dt.float8e4
EXP = mybir.ActivationFunctionType.Exp
DR = mybir.MatmulPerfMode.DoubleRow

ts = bass.ts


def build_kernel(loop=1):
    """Build + compile the single-core program (SPMD across 8 cores).

    loop > 1 unrolls the whole kernel body that many times in one NEFF; used
    by test.py to measure marginal (steady-state) HW time per execution.
    """
    nc = bacc.Bacc("TRN2", target_bir_lowering=False, debug=False)

    src_d = nc.dram_tensor("source", [C, N], F32, kind="ExternalInput").ap()
    gui_d = nc.dram_tensor("guidance", [C, N], F32, kind="ExternalInput").ap()
    wq_d = nc.dram_tensor("Wq", [CR, C], F32, kind="ExternalInput").ap()
    wk_d = nc.dram_tensor("Wk", [CR, C], F32, kind="ExternalInput").ap()
    wv_d = nc.dram_tensor("Wv", [C, C], F32, kind="ExternalInput").ap()
    bv_d = nc.dram_tensor("bv", [C], F32, kind="ExternalInput").ap()
    g_d = nc.dram_tensor("gamma", [1], F32, kind="ExternalInput").ap()
    out_d = nc.dram_tensor("out", [C, N], F32, kind="ExternalOutput").ap()

    with tile.TileContext(nc) as tc:
        for it in range(loop):
            with ExitStack() as ctx:
                _body(ctx, tc, src_d, gui_d, wq_d, wk_d, wv_d, bv_d, g_d,
                      out_d, sfx=f"_{it}")
    nc.compile()
    return nc


def _body(ctx, tc, src_d, gui_d, wq_d, wk_d, wv_d, bv_d, g_d, out_d, sfx=""):
    nc = tc.nc

    consts = ctx.enter_context(tc.tile_pool(name="consts" + sfx, bufs=1))
    big = ctx.enter_context(tc.tile_pool(name="big" + sfx, bufs=1))

    # ---- persistent SBUF tensors ----
    # f32r-typed: DMA'd fp32 bits consumed directly by f32r matmuls (the
    # BIR verifier requires the memory location itself to be fp32r-typed)
    src_sb = big.tile([128, CCH, N], F32R)   # residual + Q-proj operand
    gui_sb = big.tile([128, CCH, N], F32R)   # K/V-proj operand
    # Q/K with q duplicated onto partitions 64..127 for PE row-tiling.
    QQ = big.tile([128, N], BF16)
    KK = big.tile([128, N], BF16)
    VT = big.tile([128, MCH, C], F8)         # [m%128, m//128, c] = V^T, fp8

    # ---- weights / constants ----
    wq_sb = consts.tile([CR, C], F32)
    wk_sb = consts.tile([CR, C], F32)
    wv_sb = consts.tile([128, CCH, C], F32)  # [c%128, c//128, ch]
    bv2 = consts.tile([128, 2, C], F32)      # bv broadcast, duplicated per pair
    g128 = consts.tile([128, 1], F32)
    ones8 = consts.tile([128, 2, MT], F8)    # fp8 DoubleRow row-sum operand
    ident = consts.tile([128, 128], F32)
    shift = consts.tile([128, 1], F32)       # exp bias: -EXP_SHIFT

    # Activations stream in 1024-column blocks on the SP HWDGE queue,
    # ordered by first use (DMA bandwidth is the serial resource): gui block
    # 0 feeds the first K/V projections, src block 0 the first Q projection,
    # later gui blocks feed the K chunks streamed inside the first attention
    # chunk. Small weights ride the otherwise-idle ACT HWDGE queue.
    gui_r = gui_d.rearrange("(t p) n -> t p n", p=128)
    src_r = src_d.rearrange("(t p) n -> t p n", p=128)
    NB = N // 4

    def block(dst, rr, b):
        for t in range(CCH):
            nc.sync.dma_start(out=dst[:, t, ts(b, NB)],
                              in_=rr[t][:, ts(b, NB)].bitcast(F32R))

    block(gui_sb, gui_r, 0)
    block(src_sb, src_r, 0)
    block(gui_sb, gui_r, 1)
    block(gui_sb, gui_r, 2)
    block(src_sb, src_r, 1)
    block(gui_sb, gui_r, 3)
    block(src_sb, src_r, 2)
    block(src_sb, src_r, 3)
    nc.scalar.dma_start(out=wq_sb[:], in_=wq_d)
    nc.scalar.dma_start(out=wk_sb[:], in_=wk_d)
    wv_r = wv_d.rearrange("(t p) c -> t p c", p=128)
    for t in range(CCH):
        nc.scalar.dma_start(out=wv_sb[:, t, :], in_=wv_r[t])
    for h in range(2):
        nc.scalar.dma_start(out=bv2[:, h, :], in_=bv_d.unsqueeze(0).to_broadcast([128, C]))
    nc.scalar.dma_start(out=g128[:], in_=g_d.to_broadcast([128, 1]))

    nc.vector.memset(ones8[:], 1.0)
    nc.vector.memset(shift[:], -EXP_SHIFT)
    make_identity(nc, ident[:])

    # ---- transpose weights on the PE (fp32, consumed as f32r) ----
    # wqt2/wkt2: [c-chunk 128, q duplicated to 128]; wvt: [ch, c] = Wv^T
    wqt2 = consts.tile([128, CCH, 128], F32R)
    wkt2 = consts.tile([128, CCH, 128], F32R)
    wvt = consts.tile([128, CCH, C], F32R)

    with tc.tile_pool(name="tp_psum" + sfx, bufs=2, space=bass.MemorySpace.PSUM) as tpp:
        for t in range(CCH):
            for w_sb, w_t2 in ((wk_sb, wkt2), (wq_sb, wqt2)):
                p = tpp.tile([128, CR], F32, tag="tp")
                nc.tensor.transpose(p[:], w_sb[:, ts(t, 128)], ident[:CR, :CR])
                nc.vector.tensor_copy(w_t2[:, t, 0:CR], p[:])
                nc.vector.tensor_copy(w_t2[:, t, CR:128], p[:])
            for j in range(CCH):
                # wvt[:, t, j*128:+128] = Wv[j*128:+128, t*128:+128]^T
                p = tpp.tile([128, 128], F32, tag="tp")
                nc.tensor.transpose(p[:], wv_sb[:, j, ts(t, 128)], ident[:])
                nc.vector.tensor_copy(wvt[:, t, ts(j, 128)], p[:])


    # s_ps is opened before the projection pool: the sm denominator ring is
    # also the Q-projection ring (alternating slots), and the c0 Q projection
    # is interleaved with the K/V projections below.
    s_ps = ctx.enter_context(
        tc.tile_pool(name="s_psum" + sfx, bufs=2, space=bass.MemorySpace.PSUM))

    # ---- prologue projections: K chunks 0-1 and Q chunk 0 only ----
    # Everything else (K chunks 2-7, all V pairs, Q chunks 1-7) streams
    # inside the attention loop so the first exp fires ~9us in, right after
    # the first DMA blocks land.
    qp0 = s_ps.tile([128, NT], F32, tag="s")
    with tc.tile_pool(name="proj_psum" + sfx, bufs=2, space=bass.MemorySpace.PSUM) as pp:
        for i in range(2):
            kp = pp.tile([128, NT], F32, tag="qk")
            for t in range(CCH):
                nc.tensor.matmul(kp[:], wkt2[:, t, :],
                                 gui_sb[:, t, ts(i, NT)],
                                 start=(t == 0), stop=(t == CCH - 1))
            nc.vector.tensor_copy(KK[:, ts(i, NT)], kp[:])
        for t in range(CCH):
            nc.tensor.matmul(qp0[:], wqt2[:, t, :],
                             src_sb[:, t, ts(0, NT)],
                             start=(t == 0), stop=(t == CCH - 1))
        nc.vector.tensor_copy(QQ[:, ts(0, NT)], qp0[:])

    # ---- attention main loop ----
    e_ps = ctx.enter_context(
        tc.tile_pool(name="e_psum" + sfx, bufs=2, space=bass.MemorySpace.PSUM))
    o_ps = ctx.enter_context(
        tc.tile_pool(name="o_psum" + sfx, bufs=2, space=bass.MemorySpace.PSUM))
    e_sb = ctx.enter_context(tc.tile_pool(name="e_sb" + sfx, bufs=4))
    fin = ctx.enter_context(tc.tile_pool(name="fin" + sfx, bufs=2))
    o_sb = ctx.enter_context(tc.tile_pool(name="o_sb" + sfx, bufs=4))

    out_r = out_d.rearrange("(t p) n -> t p n", p=128)

    def q_proj(i):
        qp = s_ps.tile([128, NT], F32, tag="s")
        for t in range(CCH):
            nc.tensor.matmul(qp[:], wqt2[:, t, :],
                             src_sb[:, t, ts(i, NT)],
                             start=(t == 0), stop=(t == CCH - 1))
        nc.vector.tensor_copy(QQ[:, ts(i, NT)], qp[:])

    def v_pair(jj):
        # V pair rides the e_ps ring: each half of the [128, 2, NT] tile is
        # one PSUM bank; one DVE drain per pair folds in the bias (fp8 out)
        vp = e_ps.tile([128, 2, NT], F32, tag="e")
        for h in range(2):
            for t in range(CCH):
                nc.tensor.matmul(vp[:, h, 0:C],
                                 gui_sb[:, t, ts(2 * jj + h, MT)],
                                 wvt[:, t, :],
                                 start=(t == 0), stop=(t == CCH - 1))
        nc.vector.tensor_add(VT[:, 2 * jj:2 * jj + 2, :], vp[:, :, 0:C], bv2[:])

    def k_proj(c):
        kp = e_ps.tile([128, 2, NT], F32, tag="e")
        for t in range(CCH):
            nc.tensor.matmul(kp[:, 0, :], wkt2[:, t, :],
                             gui_sb[:, t, ts(c, NT)],
                             start=(t == 0), stop=(t == CCH - 1))
        nc.vector.tensor_copy(KK[:, ts(c, NT)], kp[:, 0, :])

    v_pair(0)
    v_pair(1)

    for i in range(NCH):
        o0 = o_ps.tile([128, NT], F32, tag="o")
        o1 = o_ps.tile([128, NT], F32, tag="o")
        sm = s_ps.tile([128, NT], F32, tag="s")

        def energy_pair(jj):
            # two bf16 energy matmuls into one [128, 2, NT] PSUM pair tile,
            # row-group halves 0-63 / 64-127 for PE row-tiling concurrency
            ep = e_ps.tile([128, 2, NT], F32, tag="e")
            for h in range(2):
                b0 = CR * h
                nc.tensor.matmul(ep[:, h, :], KK[b0:b0 + CR, ts(2 * jj + h, MT)],
                                 QQ[b0:b0 + CR, ts(i, NT)],
                                 start=True, stop=True, tile_position=(b0, 0))
            return ep

        ep = energy_pair(0)
        for jj in range(MPAIR):
            ee = e_sb.tile([128, 2, NT], F8, tag="ee")
            nc.scalar.activation(ee[:], ep[:], EXP, bias=shift[:])
            if jj + 1 < MPAIR:
                ep = energy_pair(jj + 1)  # keep PE one pair ahead of ACT
            if i == 0:
                # stream the remaining projections through the first chunk
                if jj % 2 == 0 and jj // 2 + 2 < NCH:
                    k_proj(jj // 2 + 2)
                if jj + 2 < MPAIR:
                    v_pair(jj + 2)
            if jj == 8 and i + 1 < NCH:
                q_proj(i + 1)  # JIT projection of the next query chunk
            first, last = jj == 0, jj == MPAIR - 1
            vpair = VT[:, 2 * jj:2 * jj + 2, :]
            nc.tensor.matmul(o0[:], vpair[:, :, 0:128], ee[:],
                             start=first, stop=last, perf_mode=DR)
            nc.tensor.matmul(o1[:], vpair[:, :, 128:256], ee[:],
                             start=first, stop=last, perf_mode=DR)
            nc.tensor.matmul(sm[:], ones8[:], ee[:],
                             start=first, stop=last, perf_mode=DR)

        # out = o * (gamma / sum) + src
        rsg = fin.tile([128, NT], F32, tag="rsg")
        nc.vector.reciprocal(rsg[:], sm[:])
        nc.vector.tensor_scalar_mul(rsg[:], rsg[:], g128[:])
        # last chunk: strip-mine so the output DMAs overlap the DVE tail,
        # alternating HWDGE queues to pipeline the DMA issue cost
        nstrip = 2 if i == NCH - 1 else 1
        st = NT // nstrip
        for t, op in enumerate((o0, o1)):
            ot = o_sb.tile([128, NT], F32, tag="ot")
            for u in range(nstrip):
                sl = slice(u * st, (u + 1) * st)
                nc.vector.tensor_mul(ot[:, sl], op[:, sl], rsg[:, sl])
                nc.vector.tensor_add(ot[:, sl], ot[:, sl],
                                     src_sb[:, t, ts(i, NT)][:, sl].bitcast(F32))
                q = nc.scalar if (t + u) % 2 else nc.sync
                q.dma_start(out=out_r[t][:, ts(i, NT)][:, sl], in_=ot[:, sl])


_NC_CACHE = []


def _get_nc():
    if not _NC_CACHE:
        _NC_CACHE.append(build_kernel())
    return _NC_CACHE[0]


def make_in_maps(**inputs):
    f = lambda a: np.ascontiguousarray(np.asarray(a, dtype=np.float32))
    src = f(inputs["source"]).reshape(B, C, N)
    gui = f(inputs["guidance"]).reshape(B, C, N)
    shared = {
        "Wq": f(inputs["Wq"]),
        "Wk": f(inputs["Wk"]),
        "Wv": f(inputs["Wv"]),
        "bv": f(inputs["bv"]),
        "gamma": f(inputs["gamma"]),
    }
    return [dict(source=src[b], guidance=gui[b], **shared) for b in range(B)]


def kernel(**inputs) -> np.ndarray:
    nc = _get_nc()
    res = run_bass_kernel_spmd(nc, make_in_maps(**inputs),
                               core_ids=list(range(N_CORES)))
    out = np.stack([res.results[b]["out"] for b in range(B)])
    return out.reshape(B, C, H, W).astype(np.float32)
